# revision 102
# baseline (speedup 1.0000x reference)
# Causal self-attention kernel for 8 Trainium2 NeuronCores (Bass/Tile).
#
# Problem: x:(2,4096,768) f32, 12 heads, head_dim 64, causal mask, torch-Linear
# Q/K/V/out projections. out = softmax(QK^T/8, causal) V @ Wp^T + biases.
#
# Sharding: core i computes batch b=i//4, head group hg=i%4 (heads 3hg..3hg+2).
#   Prep: PE-transpose x_b and the weight slices to contraction-major bf16.
#   QKV:  Q^T,K^T (d-major) and V (row-major with an appended ones column).
#   Attention (per head, per 1024-wide query group): S^T = K_chunk Q^T on PE,
#     P^T = exp(S^T/8) on ACT (causal via column trim + 128x128 triangle mask),
#     PSUM-accumulate [V|1]^T P^T -> (A^T ; rowsum); divide by rowsum.
#   One AllToAll per head re-shards A^T from head-split to query-column-split
#     (part j = A^T columns [512j,512j+512)), overlapping communication with
#     the remaining heads' attention.
#   Proj: each core projects its 1024 rows (512 per batch) against Wp^T in two
#   accumulation passes (heads 0,1 early - overlaps attention; head 2 after
#     the last collective).
# All PSUM pools are open for the whole kernel (static banks), so phases
# overlap freely under Tile dependency scheduling.
# Host only slices inputs and concatenates the 8 disjoint output row blocks.

import numpy as np
import ml_dtypes

import concourse.bass as bass  # noqa: F401
import concourse.mybir as mybir
import concourse.tile as tile
from concourse import bacc
from concourse.bass_utils import run_bass_kernel_spmd

F32 = mybir.dt.float32
BF16 = mybir.dt.bfloat16

B, T, C, H, D = 2, 4096, 768, 12, 64
NCORES = 8
GROUPS = 4              # cores per batch
HPC = H // GROUPS       # 3 heads per core
JC = HPC * D            # 192 projection columns per core
P = 128
CCHUNKS = C // P        # 6 contraction chunks
RCHUNKS = T // P        # 32 row chunks of the batch
QCW = 512               # a2a part width (psum bank = 512 f32)
NQC = T // QCW          # 8
QGW = 1024              # attention query-group width (wide ACT ops)
NQG = T // QGW          # 4
ROWS_OUT = T // GROUPS  # 1024 output rows per core
SCALE = 1.0 / 8.0       # 1/sqrt(64)

_CACHE: dict = {}
LAST_RESULTS = None


def _build(debug_stage=None):
    nc = bacc.Bacc("TRN2", target_bir_lowering=False, debug=False,
                   num_devices=NCORES)

    xb = nc.dram_tensor("xb", [T, C], F32, kind="ExternalInput").ap()
    wq = nc.dram_tensor("wq", [JC, C], F32, kind="ExternalInput").ap()
    wk = nc.dram_tensor("wk", [JC, C], F32, kind="ExternalInput").ap()
    wv = nc.dram_tensor("wv", [JC, C], F32, kind="ExternalInput").ap()
    wp = nc.dram_tensor("wp", [C, C], F32, kind="ExternalInput").ap()
    bq = nc.dram_tensor("bq", [JC], F32, kind="ExternalInput").ap()
    bk = nc.dram_tensor("bk", [JC], F32, kind="ExternalInput").ap()
    bv = nc.dram_tensor("bv", [JC], F32, kind="ExternalInput").ap()
    bp = nc.dram_tensor("bp", [C], F32, kind="ExternalInput").ap()
    out = nc.dram_tensor("out_part", [ROWS_OUT, C], F32,
                         kind="ExternalOutput").ap()

    ident_d = nc.inline_tensor(np.eye(P, dtype=np.float32),
                               name="ident_const").ap()
    # tri[k, q] = 1 if k <= q (valid causal entries of a diagonal S^T block)
    tri_d = nc.inline_tensor(
        np.triu(np.ones((P, P), dtype=ml_dtypes.bfloat16)),
        name="tri_const").ap()

    with tile.TileContext(nc) as tc, \
         tc.tile_pool(name="persist", bufs=1) as persist, \
         tc.tile_pool(name="prep", bufs=3) as prep, \
         tc.tile_pool(name="att_sb", bufs=8) as att_sb, \
         tc.tile_pool(name="div_sb", bufs=3) as div_sb, \
         tc.tile_pool(name="div_dram", bufs=3, space="DRAM") as div_dram, \
         tc.tile_pool(name="a2a_dram", bufs=1, space="DRAM") as a2a_dram, \
         tc.tile_pool(name="proj_sb", bufs=4) as proj_sb:

        def ptile(shape, dtype, name):
            return persist.tile(shape, dtype, name=name, tag=name)

        # ---------- persistent SBUF tensors ----------
        identf = ptile([P, P], F32, name="identf")
        trimask = ptile([P, P], BF16, name="trimask")
        nc.sync.dma_start(identf, ident_d)
        nc.sync.dma_start(trimask, tri_d)

        xbT_all = ptile([P, CCHUNKS, T], BF16, name="xbT_all")
        xbT = [xbT_all[:, cc, :] for cc in range(CCHUNKS)]
        wqT_all = ptile([P, CCHUNKS, JC], BF16, name="wqT_all")
        wqT = [wqT_all[:, cc, :] for cc in range(CCHUNKS)]
        wkT_all = ptile([P, CCHUNKS, JC], BF16, name="wkT_all")
        wkT = [wkT_all[:, cc, :] for cc in range(CCHUNKS)]
        wvT_all = ptile([P, CCHUNKS, JC], BF16, name="wvT_all")
        wvT = [wvT_all[:, cc, :] for cc in range(CCHUNKS)]
        # wpT chunks permuted h_local-major: chunk k = h_local*2 + sp holds
        # c_in rows for (h_local, senders 2sp and 2sp+1); head-2 chunks last
        # so the output projection can start before the final collective.
        wpT_all = ptile([P, CCHUNKS, C], BF16, name="wpT_all")
        wpT = [wpT_all[:, cc, :] for cc in range(CCHUNKS)]
        qt_a = ptile([P, T], BF16, name="qt_a")    # heads 0,1 (rows 2*D)
        kt_a = ptile([P, T], BF16, name="kt_a")
        qt_b = ptile([D, T], BF16, name="qt_b")    # head 2
        kt_b = ptile([D, T], BF16, name="kt_b")
        vones = ptile([P, RCHUNKS, HPC, D + 1], BF16, name="vones")
        # agT[b2*6+k]: rows 0:64 = (h_local=k//2, sender 2*(k%2)),
        #              rows 64:128 = sender 2*(k%2)+1; columns = the core's
        # 512 query rows of batch b2.
        agT_all = ptile([P, 2 * CCHUNKS, QCW], BF16, name="agT_all")
        agT = [agT_all[:, cc, :] for cc in range(2 * CCHUNKS)]

        bqa = ptile([P, 1], F32, name="bqa")
        bqb = ptile([D, 1], F32, name="bqb")
        bka = ptile([P, 1], F32, name="bka")
        bkb = ptile([D, 1], F32, name="bkb")
        bv_bc = ptile([P, JC], F32, name="bv_bc")
        bp_bc = ptile([P, C], F32, name="bp_bc")
        nc.sync.dma_start(bqa, bq[0:P][:, None])
        nc.sync.dma_start(bqb, bq[P:JC][:, None])
        nc.sync.dma_start(bka, bk[0:P][:, None])
        nc.sync.dma_start(bkb, bk[P:JC][:, None])
        nc.sync.dma_start(bv_bc, bv[None, :].to_broadcast((P, JC)))
        nc.sync.dma_start(bp_bc, bp[None, :].to_broadcast((P, C)))

        nc.gpsimd.memset(vones[:, :, :, D:D + 1], 1.0)

        a2a_in = [a2a_dram.tile([NCORES, D, QCW], BF16, name=f"a2a_in{h}",
                                tag=f"a2a_in{h}") for h in range(HPC)]
        a2a_out = [a2a_dram.tile([NCORES * D, QCW], BF16, name=f"a2a_out{h}",
                                 tag=f"a2a_out{h}") for h in range(HPC)]

        # ---------- prep: PE f32 transposes, cast on copyback --------------
        prep_ps = tc.alloc_tile_pool(name="prep_ps", bufs=6, space="PSUM")
        tp_ctr = [0]

        def copyback(dst_ap, src_ap):
            # alternate DVE / ACT for the psum->sbuf cast copies
            if tp_ctr[0] % 2:
                nc.scalar.copy(dst_ap, src_ap)
            else:
                nc.vector.tensor_copy(dst_ap, src_ap)
            tp_ctr[0] += 1

        def transpose_in(dst, dst_col0, src_sb):
            # src_sb: (j<=128, w<=128) f32 -> bf16 dst[:w, col0:col0+j]
            j, w = src_sb.shape
            pst = prep_ps.tile([P, P], F32, name="pst", tag="pst")
            nc.tensor.transpose(pst[:w, :j], src_sb, identf[:j, :j])
            copyback(dst[:w, dst_col0:dst_col0 + j], pst[:w, :j])

        # weights wq/wk/wv: (192, 768) -> wT chunks (128, 192)
        for w_ap, wT in ((wq, wqT), (wk, wkT), (wv, wvT)):
            for part, rows in ((0, P), (P, D)):
                wn = prep.tile([rows, C], F32, name="wn", tag=f"wn{rows}")
                nc.sync.dma_start(wn, w_ap[part:part + rows, :])
                for cc in range(CCHUNKS):
                    transpose_in(wT[cc], part, wn[:, cc * P:(cc + 1) * P])
        # x: (4096, 768) -> xbT chunks (128, 4096)
        for rc in range(RCHUNKS):
            xn = prep.tile([P, C], F32, name="xn", tag="wn128")
            nc.sync.dma_start(xn, xb[rc * P:(rc + 1) * P, :])
            for cc in range(CCHUNKS):
                transpose_in(xbT[cc], rc * P, xn[:, cc * P:(cc + 1) * P])
        # wp (needed only by the late output projection): permuted wpT chunks
        for jc in range(CCHUNKS):
            wn = prep.tile([P, C], F32, name="wpn", tag="wn128")
            nc.sync.dma_start(wn, wp[jc * P:(jc + 1) * P, :])
            for k in range(CCHUNKS):
                h_local, sp = k // 2, k % 2
                for half in range(2):
                    hh = HPC * (2 * sp + half) + h_local
                    pst = prep_ps.tile([P, P], F32, name="pst", tag="pst")
                    src = wn[:, D * hh:D * (hh + 1)]  # (128 j, 64 c)
                    nc.tensor.transpose(pst[:D, :P], src, identf)
                    copyback(wpT[k][D * half:D * (half + 1),
                                    jc * P:(jc + 1) * P], pst[:D, :P])

        prep_ps.release()

        # ---------- QKV projections (scoped PSUM pools) ---------------------
        qkv_ps = tc.alloc_tile_pool(name="qkv_ps", bufs=1, space="PSUM")
        v_ps = tc.alloc_tile_pool(name="v_ps", bufs=2, space="PSUM")
        for qc in range(NQC):
            cs = slice(qc * QCW, (qc + 1) * QCW)
            psqa = qkv_ps.tile([P, QCW], F32, name="psqa", tag="psqa")
            psqb = qkv_ps.tile([D, QCW], F32, name="psqb", tag="psqb")
            for cc in range(CCHUNKS):
                st, sp = (cc == 0), (cc == CCHUNKS - 1)
                rhs = xbT[cc][:, cs]
                nc.tensor.matmul(psqa, wqT[cc][:, 0:P], rhs, start=st, stop=sp)
                nc.tensor.matmul(psqb, wqT[cc][:, P:JC], rhs, start=st,
                                 stop=sp)
            nc.vector.tensor_scalar_add(qt_a[:, cs], psqa, bqa)
            nc.vector.tensor_scalar_add(qt_b[:, cs], psqb, bqb)
            pska = qkv_ps.tile([P, QCW], F32, name="pska", tag="pska")
            pskb = qkv_ps.tile([D, QCW], F32, name="pskb", tag="pskb")
            for cc in range(CCHUNKS):
                st, sp = (cc == 0), (cc == CCHUNKS - 1)
                rhs = xbT[cc][:, cs]
                nc.tensor.matmul(pska, wkT[cc][:, 0:P], rhs, start=st, stop=sp)
                nc.tensor.matmul(pskb, wkT[cc][:, P:JC], rhs, start=st,
                                 stop=sp)
            nc.vector.tensor_scalar_add(kt_a[:, cs], pska, bka)
            nc.vector.tensor_scalar_add(kt_b[:, cs], pskb, bkb)
        for rc in range(RCHUNKS):
            psv = v_ps.tile([P, JC], F32, name="psv", tag="psv")
            for cc in range(CCHUNKS):
                nc.tensor.matmul(psv, xbT[cc][:, rc * P:(rc + 1) * P],
                                 wvT[cc], start=(cc == 0),
                                 stop=(cc == CCHUNKS - 1))
            nc.vector.tensor_add(
                vones[:, rc, :, 0:D],
                psv.rearrange("p (h d) -> p h d", h=HPC),
                bv_bc.rearrange("p (h d) -> p h d", h=HPC))
        v_ps.release()
        qkv_ps.release()

        # ---------- attention + per-head AllToAll ---------------------------
        ps_s = tc.alloc_tile_pool(name="ps_s", bufs=2, space="PSUM")
        ps_o = tc.alloc_tile_pool(name="ps_o", bufs=2, space="PSUM")
        head_q = [qt_a[0:D], qt_a[D:2 * D], qt_b[0:D]]
        head_k = [kt_a[0:D], kt_a[D:2 * D], kt_b[0:D]]
        for h in range(HPC):
            qh, kh = head_q[h], head_k[h]
            for qg in range(NQG):
                pso = ps_o.tile([D + 1, QGW], F32, name="pso", tag="pso")
                nkc = (qg + 1) * (QGW // P)
                for kc in range(nkc):
                    qoff = max(0, kc * P - qg * QGW)
                    pss = ps_s.tile([P, QGW], F32, name="pss", tag="pss")
                    for sub in range(QGW // QCW):
                        lo, hi = max(qoff, sub * QCW), (sub + 1) * QCW
                        if lo >= hi:
                            continue
                        nc.tensor.matmul(
                            pss[:, lo:hi], kh[:, kc * P:(kc + 1) * P],
                            qh[:, qg * QGW + lo:qg * QGW + hi],
                            start=True, stop=True)
                    pT = att_sb.tile([P, QGW], BF16, name="pT", tag="pT")
                    nc.scalar.activation(pT[:, qoff:QGW], pss[:, qoff:QGW],
                                         mybir.ActivationFunctionType.Exp,
                                         scale=SCALE)
                    if kc >= qg * (QGW // P):
                        nc.vector.tensor_mul(pT[:, qoff:qoff + P],
                                             pT[:, qoff:qoff + P], trimask)
                    for sub in range(QGW // QCW):
                        lo, hi = max(qoff, sub * QCW), (sub + 1) * QCW
                        if lo >= hi:
                            continue
                        nc.tensor.matmul(
                            pso[:, lo:hi], vones[:, kc, h, :], pT[:, lo:hi],
                            start=(kc == 0), stop=(kc == nkc - 1))
                recip = div_sb.tile([1, QGW], F32, name="recip", tag="recip")
                nc.vector.reciprocal(recip, pso[D:D + 1, :])
                araw = div_sb.tile([D, QGW], BF16, name="araw", tag="araw")
                nc.vector.tensor_copy(araw, pso[0:D, :])  # frees pso early
                # partition-broadcast must bounce through DRAM
                rdram = div_dram.tile([1, QGW], F32, name="rdram", tag="rdram")
                nc.sync.dma_start(rdram, recip)
                rbc = div_sb.tile([D, QGW], F32, name="rbc", tag="rbc")
                nc.sync.dma_start(rbc, rdram.to_broadcast((D, QGW)))
                atile = div_sb.tile([D, QGW], BF16, name="atile", tag="atile")
                nc.vector.tensor_mul(atile, araw, rbc)
                for half in range(2):
                    nc.sync.dma_start(
                        a2a_in[h][2 * qg + half, :, :],
                        atile[:, half * QCW:(half + 1) * QCW])
            # per-head AllToAll: receiver j gets (8, 64, 512); rows
            # 64*sender..+64 = head (3*(sender%4)+h) of batch sender//4,
            # A^T columns [512j, 512j+512).
            nc.gpsimd.collective_compute(
                "AllToAll", mybir.AluOpType.bypass,
                replica_groups=[list(range(NCORES))],
                ins=[a2a_in[h].opt()], outs=[a2a_out[h].opt()])
            # assemble this head's agT slices as soon as its collective lands
            for b2 in range(2):
                for sp in range(2):
                    k = 2 * h + sp
                    for half in range(2):
                        sender = 4 * b2 + 2 * sp + half
                        nc.sync.dma_start(
                            agT[b2 * CCHUNKS + k][D * half:D * (half + 1), :],
                            a2a_out[h][sender * D:(sender + 1) * D, :])
        ps_o.release()
        ps_s.release()

        # ---------- output projection ---------------------------------------
        ps_pj = tc.alloc_tile_pool(name="ps_pj", bufs=4, space="PSUM")
        for b2 in range(2):
            for rc in range(QCW // P):
                pa = ps_pj.tile([P, QCW], F32, name="pa", tag="pa")
                pb = ps_pj.tile([P, C - QCW], F32, name="pb", tag="pb")
                for k in range(CCHUNKS):
                    lhsT = agT[b2 * CCHUNKS + k][:, rc * P:(rc + 1) * P]
                    st, sp = (k == 0), (k == CCHUNKS - 1)
                    nc.tensor.matmul(pa, lhsT, wpT[k][:, 0:QCW], start=st,
                                     stop=sp)
                    nc.tensor.matmul(pb, lhsT, wpT[k][:, QCW:C], start=st,
                                     stop=sp)
                osb2 = proj_sb.tile([P, C], F32, name="osb2", tag="osb2")
                nc.vector.tensor_add(osb2[:, 0:QCW], pa, bp_bc[:, 0:QCW])
                nc.vector.tensor_add(osb2[:, QCW:C], pb, bp_bc[:, QCW:C])
                row0 = b2 * QCW + rc * P
                eng = nc.sync if rc % 2 else nc.scalar
                eng.dma_start(out[row0:row0 + P, :], osb2)
        ps_pj.release()

    nc.compile()
    return nc


def kernel(**inputs) -> np.ndarray:
    global LAST_RESULTS
    x = np.ascontiguousarray(np.asarray(inputs["x"], dtype=np.float32))
    Wq = np.ascontiguousarray(np.asarray(inputs["Wq"], dtype=np.float32))
    Wk = np.ascontiguousarray(np.asarray(inputs["Wk"], dtype=np.float32))
    Wv = np.ascontiguousarray(np.asarray(inputs["Wv"], dtype=np.float32))
    Wp = np.ascontiguousarray(np.asarray(inputs["Wp"], dtype=np.float32))
    bq = np.ascontiguousarray(np.asarray(inputs["bq"], dtype=np.float32))
    bk = np.ascontiguousarray(np.asarray(inputs["bk"], dtype=np.float32))
    bv = np.ascontiguousarray(np.asarray(inputs["bv"], dtype=np.float32))
    bp = np.ascontiguousarray(np.asarray(inputs["bp"], dtype=np.float32))

    if "nc" not in _CACHE:
        _CACHE["nc"] = _build()
    nc = _CACHE["nc"]

    in_maps = []
    for core in range(NCORES):
        b = core // GROUPS
        hg = core % GROUPS
        js = slice(JC * hg, JC * (hg + 1))
        in_maps.append({
            "xb": np.ascontiguousarray(x[b]),
            "wq": np.ascontiguousarray(Wq[js]),
            "wk": np.ascontiguousarray(Wk[js]),
            "wv": np.ascontiguousarray(Wv[js]),
            "wp": Wp,
            "bq": np.ascontiguousarray(bq[js]),
            "bk": np.ascontiguousarray(bk[js]),
            "bv": np.ascontiguousarray(bv[js]),
            "bp": bp,
        })

    res = run_bass_kernel_spmd(nc, in_maps, core_ids=list(range(NCORES)))
    LAST_RESULTS = res

    out = np.empty((B, T, C), dtype=np.float32)
    for core in range(NCORES):
        part = res.results[core]["out_part"]
        out[0, core * QCW:(core + 1) * QCW, :] = part[:QCW]
        out[1, core * QCW:(core + 1) * QCW, :] = part[QCW:]
    return out
